# revision 2
# baseline (speedup 1.0000x reference)
"""Causal MHA (B=2, N=2048, D=1024, H=16) on 8 NeuronCores via Bass/Tile. v3.

Sharding: core c = (b, g): b = c // 4 (batch), g = c % 4 (head group of 4
heads = 256 features). Each core computes its Q/K/V projections, causal
attention for its 4 heads, and a partial output projection (its 256 rows of
Wo). The host sums the 4 partials per batch and adds bo + bv @ Wo (softmax
rows sum to 1, so the V bias contributes exactly bv @ Wo to every row).

All matmuls bf16. Layout is feature-major (features on partitions, sequence
free). K is stored per-head zero-padded to 128 contraction rows: K=64
matmuls measure ~420ns vs ~250ns for K=128 at free=512, so a zero half +
both heads' Q in the moving operand is strictly faster. S and P@V are
trimmed to the causal region at 128-key granularity; exp covers exactly the
written region via one strided AP per tile.

Engine assignment rules learned from microbenchmarks:
- every PSUM evacuation runs on ACT (DVE touching PSUM while the PE is
  streaming matmuls serializes the PE: 432ns/mm vs 246ns/mm);
- DVE handles only SBUF-side work (causal masks, reciprocal, normalize);
- softmax reciprocal broadcast bounces through DRAM on the gpsimd DMA queue
  so it never queues behind the big activation loads.
PSUM is organized as four [128,1024] double-bank tiles (tags x2/y2): the
projection passes pack two 512-column groups per tile and evacuate with one
wide ACT op; attention uses x2 for score tiles and y2 for the P@V
accumulators and O-proj tiles.
"""

import numpy as np
import ml_dtypes

import concourse.bass as bass
import concourse.bacc as bacc
import concourse.mybir as mybir
from concourse.tile import TileContext
from concourse.bass_utils import run_bass_kernel_spmd

F32 = mybir.dt.float32
F32R = mybir.dt.float32r
BF16 = mybir.dt.bfloat16
AF = mybir.ActivationFunctionType
NPBF16 = ml_dtypes.bfloat16

B, N, D, H, DH = 2, 2048, 1024, 16, 64
NCORES = 8
GROUPS = 4
HPC = H // GROUPS     # 4 heads per core
FS = HPC * DH         # 256
P = 128
NDT = N // 128        # 16
NSS = N // 512        # 4
DT = D // 128         # 8
FT = FS // 128        # 2

_CACHE = {}


def _build(repeat=1, phases="all"):
    nc = bacc.Bacc("TRN2", target_bir_lowering=False, debug=False)

    xqT = nc.dram_tensor("xqT", [D, N], BF16, kind="ExternalInput")
    xkvT = nc.dram_tensor("xkvT", [D, N], BF16, kind="ExternalInput")
    wq = nc.dram_tensor("wq", [D, FS], BF16, kind="ExternalInput")
    wk = nc.dram_tensor("wk", [D, FS], BF16, kind="ExternalInput")
    wv = nc.dram_tensor("wv", [D, FS], BF16, kind="ExternalInput")
    wo = nc.dram_tensor("wo", [FS, D], BF16, kind="ExternalInput")
    bq = nc.dram_tensor("bq", [FS], F32, kind="ExternalInput")
    bk = nc.dram_tensor("bk", [FS], F32, kind="ExternalInput")
    masks = nc.dram_tensor("masks", [P, P], BF16, kind="ExternalInput")
    out = nc.dram_tensor("out_p", [N, D], BF16, kind="ExternalOutput")

    with TileContext(nc) as tc:
        with (
            tc.tile_pool(name="const", bufs=1) as cp,
            tc.tile_pool(name="xt", bufs=1) as xp,
            tc.tile_pool(name="acts", bufs=1) as ap_,
            tc.tile_pool(name="ps", bufs=2, space="PSUM") as psp,
            tc.tile_pool(name="pt", bufs=3) as ptp,
            tc.tile_pool(name="small", bufs=2) as smp,
            tc.tile_pool(name="osb", bufs=3) as osp,
            tc.tile_pool(name="dsc", bufs=4, space="DRAM") as dsp,
        ):
            wq_sb = cp.tile([P, DT, FS], BF16, tag="wq")
            wk_sb = cp.tile([P, DT, FS], BF16, tag="wk")
            wv_sb = cp.tile([P, DT, FS], BF16, tag="wv")
            wo_sb = cp.tile([P, FT, D], BF16, tag="wo")
            bqk_sb = cp.tile([P, 2, 2], F32, tag="bqk")
            tri_sb = cp.tile([P, P], BF16, tag="mask")

            nc.sync.dma_start(out=wq_sb, in_=wq.ap().rearrange("(t p) f -> p t f", p=P))
            nc.sync.dma_start(out=wk_sb, in_=wk.ap().rearrange("(t p) f -> p t f", p=P))
            nc.sync.dma_start(out=wv_sb, in_=wv.ap().rearrange("(t p) f -> p t f", p=P))
            nc.sync.dma_start(out=wo_sb, in_=wo.ap().rearrange("(t p) f -> p t f", p=P))
            nc.sync.dma_start(out=bqk_sb[:, 0, :], in_=bk.ap().rearrange("(t p) -> p t", p=P))
            nc.sync.dma_start(out=bqk_sb[:, 1, :], in_=bq.ap().rearrange("(t p) -> p t", p=P))
            nc.sync.dma_start(out=tri_sb, in_=masks.ap())

            # per-head K, zero-padded to 128 contraction rows (padding rows
            # memset once here; the body only ever writes the data rows)
            ktp = [ap_.tile([P, N], BF16, tag=f"ktp{h}", name=f"ktp{h}") for h in range(HPC)]
            for h in range(HPC):
                zrows = slice(64, 128) if h % 2 == 0 else slice(0, 64)
                nc.vector.memset(ktp[h][zrows, :], 0.0)
            qt_all = [ap_.tile([P, N], BF16, tag=f"qt{f}", name=f"qt{f}") for f in range(FT)]
            ot_all = [ap_.tile([P, N], BF16, tag=f"ot{f}", name=f"ot{f}") for f in range(FT)]
            # V stored two seq-tiles per SBUF tile: [seq-part, pair, head, dh+1]
            v_sb = [ap_.tile([P, 2, HPC, DH + 1], BF16, tag=f"v{sp}", name=f"v{sp}")
                    for sp in range(NDT // 2)]
            for sp in range(NDT // 2):
                nc.vector.memset(v_sb[sp][:, :, :, DH], 1.0)

            def ps2(which, name):
                return psp.tile([P, 1024], F32, tag=which, bufs=2, name=name)

            def emit_body():
                # ---- x loads (xkv first; xq overlaps the K/V passes) ----
                xkv_t, xq_t = [], []
                for d in range(DT):
                    t = xp.tile([P, N], BF16, tag=f"xkv{d}", name=f"xkv{d}")
                    nc.sync.dma_start(out=t, in_=xkvT.ap()[d * P:(d + 1) * P, :])
                    xkv_t.append(t)
                for d in range(DT):
                    t = xp.tile([P, N], BF16, tag=f"xq{d}", name=f"xq{d}")
                    nc.sync.dma_start(out=t, in_=xqT.ap()[d * P:(d + 1) * P, :])
                    xq_t.append(t)

                # ---- K pass: two 512-col groups per psum tile, one wide evac ----
                for ft in range(FT):
                    for sp in range(NSS // 2):
                        ps = ps2("xy"[(ft + sp) % 2], "ps_k")
                        for half in range(2):
                            ss = sp * 2 + half
                            for d in range(DT):
                                nc.tensor.matmul(
                                    ps[:, half * 512:(half + 1) * 512],
                                    wk_sb[:, d, ft * P:(ft + 1) * P],
                                    xkv_t[d][:, ss * 512:(ss + 1) * 512],
                                    start=(d == 0),
                                    stop=(d == DT - 1),
                                )
                        for hh in range(2):
                            h = 2 * ft + hh
                            rows = slice(hh * 64, hh * 64 + 64)
                            nc.scalar.activation(
                                ktp[h][rows, sp * 1024:(sp + 1) * 1024],
                                ps[rows, :],
                                AF.Identity,
                                bias=bqk_sb[rows, 0, ft:ft + 1],
                            )

                # ---- V pass: two seq-tiles per psum tile ----
                for sp in range(NDT // 2):
                    psv = ps2("xy"[sp % 2], "ps_v")
                    for half in range(2):
                        st = sp * 2 + half
                        for d in range(DT):
                            nc.tensor.matmul(
                                psv[:, half * 512:half * 512 + FS],
                                xkv_t[d][:, st * P:(st + 1) * P],
                                wv_sb[:, d, :],
                                start=(d == 0),
                                stop=(d == DT - 1),
                            )
                    nc.scalar.activation(
                        v_sb[sp][:, :, :, 0:DH],
                        psv.rearrange("p (j c) -> p j c", j=2)[:, :, 0:FS]
                           .rearrange("p j (h c) -> p j h c", h=HPC),
                        AF.Copy,
                    )

                # ---- Q pass ----
                for ft in range(FT):
                    for sp in range(NSS // 2):
                        ps = ps2("xy"[(ft + sp) % 2], "ps_q")
                        for half in range(2):
                            ss = sp * 2 + half
                            for d in range(DT):
                                nc.tensor.matmul(
                                    ps[:, half * 512:(half + 1) * 512],
                                    wq_sb[:, d, ft * P:(ft + 1) * P],
                                    xq_t[d][:, ss * 512:(ss + 1) * 512],
                                    start=(d == 0),
                                    stop=(d == DT - 1),
                                )
                        nc.scalar.activation(
                            qt_all[ft][:, sp * 1024:(sp + 1) * 1024],
                            ps,
                            AF.Identity,
                            bias=bqk_sb[:, 1, ft:ft + 1],
                        )

                if phases == "proj":
                    for h in range(HPC):
                        nc.sync.dma_start(
                            out=out.ap()[h * P:(h + 1) * P, :],
                            in_=ktp[h][:, 0:D],
                        )
                    for f in range(FT):
                        nc.sync.dma_start(
                            out=out.ap()[(4 + f) * P:(5 + f) * P, :],
                            in_=qt_all[f][:, 0:D],
                        )
                    for sp in range(NDT // 2):
                        nc.sync.dma_start(
                            out=out.ap()[(8 + sp) * P:(9 + sp) * P, 0:2 * HPC * (DH + 1)],
                            in_=v_sb[sp].rearrange("p j h c -> p (j h c)"),
                        )
                    return

                def emit_oproj(ss_):
                    for qt in range(4 * ss_, 4 * ss_ + 4):
                        ps_o = ps2("y", "ps_o")
                        for os_ in range(2):
                            for ft in range(FT):
                                nc.tensor.matmul(
                                    ps_o[:, os_ * 512:(os_ + 1) * 512],
                                    ot_all[ft][:, qt * P:(qt + 1) * P],
                                    wo_sb[:, ft, os_ * 512:(os_ + 1) * 512],
                                    start=(ft == 0),
                                    stop=(ft == FT - 1),
                                )
                        o_sb = osp.tile([P, D], BF16, tag="osb", name="o_sb")
                        nc.scalar.activation(o_sb, ps_o, AF.Copy)
                        nc.sync.dma_start(out=out.ap()[qt * P:(qt + 1) * P, :], in_=o_sb)

                # ---- attention (2 heads per score tile) + interleaved O-proj ----
                for ss in range(NSS):
                    n_kt = 4 * ss + 4
                    for ft in range(FT):
                        otp = ps2("y", "ps_ot")   # [dh+1, 512] x 2 heads, packed
                        for kt in range(n_kt):
                            dk = (kt - 4 * ss) * P
                            lo = max(dk, 0)
                            st2 = ps2("x", "ps_st2")
                            ptt = ptp.tile([P, 1024], BF16, tag="pt", name="ptt")
                            for hh in range(2):
                                h = 2 * ft + hh
                                nc.tensor.matmul(
                                    st2[:, hh * 512 + lo:(hh + 1) * 512],
                                    ktp[h][:, kt * P:(kt + 1) * P],
                                    qt_all[ft][:, ss * 512 + lo:(ss + 1) * 512],
                                    start=True, stop=True,
                                )
                            if lo > 0:
                                # per-head contiguous (stride-1) trimmed exp
                                for hh in range(2):
                                    nc.scalar.activation(
                                        ptt[:, hh * 512 + lo:(hh + 1) * 512],
                                        st2[:, hh * 512 + lo:(hh + 1) * 512],
                                        AF.Exp, scale=0.125,
                                    )
                            else:
                                nc.scalar.activation(ptt, st2, AF.Exp, scale=0.125)
                            if dk >= 0:
                                for hh in range(2):
                                    base = hh * 512 + dk
                                    nc.vector.tensor_mul(
                                        ptt[:, base:base + P],
                                        ptt[:, base:base + P],
                                        tri_sb,
                                    )
                            for hh in range(2):
                                nc.tensor.matmul(
                                    otp[0:DH + 1, hh * 512 + lo:(hh + 1) * 512],
                                    v_sb[kt // 2][:, kt % 2, ft * 2 + hh, :],
                                    ptt[:, hh * 512 + lo:(hh + 1) * 512],
                                    start=(kt == 0),
                                    stop=(kt == n_kt - 1),
                                    skip_group_check=True,
                                )
                        # evacuate P@V + denominators to SBUF on ACT, then
                        # normalize entirely SBUF-side on DVE
                        ob = smp.tile([DH + 1, 1024], F32, tag="ob", bufs=2, name="ob")
                        nc.scalar.activation(ob, otp[0:DH + 1, :], AF.Copy)
                        rept = smp.tile([DH + 1, 1024], F32, tag="rep_sb", bufs=2, name="rept")
                        recip = rept[DH:DH + 1, :]
                        rep_sb = rept[0:DH, :]
                        with nc.allow_low_precision(reason="softmax reciprocal"):
                            nc.vector.reciprocal(recip, ob[DH:DH + 1, :])
                        dscr = dsp.tile([1, 1024], F32, tag="dscr", name="dscr")
                        nc.gpsimd.dma_start(out=dscr, in_=recip)
                        rep_bcast = bass.AP(
                            tensor=dscr.tensor,
                            offset=dscr.offset,
                            ap=[[0, DH]] + [list(x) for x in dscr.ap[1:]],
                        )
                        nc.gpsimd.dma_start(out=rep_sb, in_=rep_bcast)
                        for hh in range(2):
                            row = hh * 64
                            nc.vector.tensor_mul(
                                ot_all[ft][row:row + 64, ss * 512:(ss + 1) * 512],
                                ob[0:DH, hh * 512:(hh + 1) * 512],
                                rep_sb[:, hh * 512:(hh + 1) * 512],
                            )

                    if phases == "proj+attn":
                        continue
                    # O-proj deferred one ss so the PE never waits on the
                    # normalize chain of the slice it is about to project
                    if ss > 0:
                        emit_oproj(ss - 1)
                    if ss == NSS - 1:
                        emit_oproj(ss)

                if phases == "proj+attn":
                    for f in range(FT):
                        nc.sync.dma_start(
                            out=out.ap()[f * P:(f + 1) * P, :],
                            in_=ot_all[f][:, 0:D],
                        )
                    return

            if repeat == 1:
                emit_body()
            else:
                with tc.For_i(0, repeat, 1):
                    emit_body()

    nc.compile()
    return nc


def _shard_inputs(x_q, x_kv, Wq, bq_, Wk, bk_, Wv, bv_, Wo, bo_):
    pp_, ff = np.meshgrid(np.arange(P), np.arange(P), indexing="ij")
    mask = (ff >= pp_).astype(NPBF16)
    xqTb = [np.ascontiguousarray(x_q[b].T.astype(NPBF16)) for b in range(B)]
    xkvTb = [np.ascontiguousarray(x_kv[b].T.astype(NPBF16)) for b in range(B)]
    in_maps = []
    for c in range(NCORES):
        b, g = c // GROUPS, c % GROUPS
        sl = slice(g * FS, (g + 1) * FS)
        in_maps.append({
            "xqT": xqTb[b],
            "xkvT": xkvTb[b],
            "wq": np.ascontiguousarray(Wq[:, sl].astype(NPBF16)),
            "wk": np.ascontiguousarray(Wk[:, sl].astype(NPBF16)),
            "wv": np.ascontiguousarray(Wv[:, sl].astype(NPBF16)),
            "wo": np.ascontiguousarray(Wo[sl, :].astype(NPBF16)),
            "bq": np.ascontiguousarray(bq_[sl]),
            "bk": np.ascontiguousarray(bk_[sl]),
            "masks": mask,
        })
    return in_maps


def kernel(x_q, x_kv, Wq, bq, Wk, bk, Wv, bv, Wo, bo):
    x_q = np.asarray(x_q, dtype=np.float32)
    x_kv = np.asarray(x_kv, dtype=np.float32)
    bv = np.asarray(bv, np.float32)
    bo = np.asarray(bo, np.float32)
    Wo_f = np.asarray(Wo, np.float32)
    if "nc" not in _CACHE:
        _CACHE["nc"] = _build()
    nc = _CACHE["nc"]
    in_maps = _shard_inputs(
        x_q, x_kv,
        np.asarray(Wq, np.float32), np.asarray(bq, np.float32),
        np.asarray(Wk, np.float32), np.asarray(bk, np.float32),
        np.asarray(Wv, np.float32), bv, Wo_f, bo,
    )
    res = run_bass_kernel_spmd(nc, in_maps, core_ids=list(range(NCORES)))
    out = np.zeros((B, N, D), dtype=np.float32)
    for c in range(NCORES):
        out[c // GROUPS] += res.results[c]["out_p"].astype(np.float32)
    # softmax rows sum to 1, so V-bias contributes exactly bv @ Wo per row
    out += (bo + bv @ Wo_f).reshape(1, 1, D)
    return out
